# revision 13
# baseline (speedup 1.0000x reference)
"""Trainium2 Bass kernel for NonLocalAttention (fused 1x1 convs + spatial softmax attention).

Reference computation (N=2, C=64, FC=64, CR=32, H=W=96, HW=9216):
    q = relu(wq @ x + bq)          [N, 32, HW]
    k = relu(wk @ fm + bk)         [N, 32, HW]
    v = relu(wa @ fm + ba)         [N, 64, HW]
    s = softmax(q^T k, axis=keys)  [N, HW, HW]
    o = s @ v^T                    [N, HW, 64]
    out = relu(wo @ [x; o^T] + bo) [N, 64, HW]

Sharding: 8 cores = batch(2) x query-rows(4).  Each core handles 2304 query
pixels of one batch element and needs the full fusionmap of that batch.

Per-core kernel (flash-style, score never goes to HBM), v2 = fp8 DoubleRow:
  - score computed TRANSPOSED: st[key, q] = k^T q via row-packed (K=32)
    matmuls, 3 key tiles at a time (G=3): groups 0,1 land in a 2-bank PSUM
    tile (psAB), group 2 in a 1-bank tile (psC).  Separate tiles so the two
    exp engines each read their OWN tile (the tile scheduler serializes
    cross-engine readers of a shared tile -- bank splits of one tile ran
    lockstep in v1).
  - exp runs CONCURRENTLY on both engines every step: one engine takes psAB
    (1024 elems/partition), the other psC (512).  ScalarE does exact LUT
    exp(x-SHIFT) -> fp8e4; VectorE does an int8 Schraudolph (y=int8(x*S8+B8);
    the int8 bit pattern read as e4m3 is ~exp(x-SHIFT), +-8% sawtooth that
    softmax normalization cancels -- measured end-to-end rel err 1.8e-4).
    ScalarE is the faster engine (1.2 vs ~1.0 elem/ns/partition) so it takes
    psAB on 2 of 3 steps.  SHIFT=1.6 keeps exp in [0.2, 235]: TRN fp8e4
    tops out at 240 (248+ = inf), and softmax is shift-invariant.
  - exp output goes to a 2-step ST super-tile [128, 6, 512] so consecutive
    key tiles sit in adjacent planes -> mm2 runs fp8 DoubleRow: each matmul
    contracts a PAIR of key tiles (256 keys) streaming 2 elems/cycle, half
    the v1 mm2 stream time.  lhsT = VT pairs [128, 2, 65]; column 64 is 1.0
    straight out of the v-conv (wa augmented with a ones column) so PSUM row
    64 accumulates the softmax denominator for free.
  - v-conv writes only cols 0:65 of each [128, 80]-strided VT slot (v1
    relu'd 128 wide; cols 65+ are never read -- garbage is confined to acc
    rows 65..127 which are never read either).
  - mm2 pair-groups lag ~2 macro-steps behind mm1 so the in-order PE queue
    never sits behind an unfinished exp; mm1 pairs are emitted back-to-back
    (halves mm1<->mm2 stationary transitions).
  - ~7us junk-matmul warm-up during the initial DMA wait locks the HAM
    clock gate at full rate (without it runs enter the loop at half PE
    clock).
  - input DMAs are spread over four engine queues (sync/scalar/vector/
    gpsimd) so descriptor generation is not serialized on one engine
    (v1: first matmul at ~16us; the DMA bytes themselves are ~2.3MB).
  - normalize: PSUM row 64 -> spread over 64 partitions by DMA, DVE
    reciprocal, gather back, broadcast by a K=1 matmul.  The LAST chunk
    instead runs the reciprocal directly on the [1, 512] row (DVE streams
    it through one lane, ~540ns) -- latency beats the two DMA round trips
    when nothing overlaps the tail.
"""

import sys

sys.path.insert(0, "/opt/trn_rl_repo")

from contextlib import ExitStack

import ml_dtypes
import numpy as np

import concourse.bacc as bacc
import concourse.bass as bass
import concourse.tile as tile
from concourse import mybir
from concourse import bass_utils

C = 64
FC = 64
CR = 32
N = 2
H = W = 96
HW = H * W            # 9216
NCORES = 8
QPC = HW // 4         # queries per core = 2304
NKT = HW // 128       # 72 key tiles
G = 3                 # row-packing group (3 key tiles concurrently)
NJ = NKT // G         # 24 key-tile groups
QCHUNKS = [(0, 512), (512, 512), (1024, 256), (1280, 512), (1792, 512)]

F32 = mybir.dt.float32
F32R = mybir.dt.float32r
BF16 = mybir.dt.bfloat16
F8 = mybir.dt.float8e4
I8 = mybir.dt.int8
DR = mybir.MatmulPerfMode.DoubleRow
EXPF = mybir.ActivationFunctionType.Exp

LOG2E = 1.4426950408889634
SHIFT = 1.6           # exp(s-SHIFT): s<=~6.5 -> exp<=~235 < fp8e4 max 240
S8 = 8.0 * LOG2E
# int8 Schraudolph bias: (7<<3) - 8*0.0579 (sawtooth centering), minus the
# folded shift.
B8C = 55.537 - SHIFT * S8


def build_bass():
    nc = bacc.Bacc(
        "TRN2", target_bir_lowering=False, debug=False, num_devices=NCORES
    )

    x_aug = nc.dram_tensor("x_aug", [C + 1, QPC], F32R, kind="ExternalInput")
    x_bf = nc.dram_tensor("x_bf", [C + 1, QPC], BF16, kind="ExternalInput")
    fm_aug = nc.dram_tensor("fm_aug", [FC + 1, HW], BF16, kind="ExternalInput")
    wq_aug = nc.dram_tensor("wq_aug", [C + 1, CR], BF16, kind="ExternalInput")
    wk_aug = nc.dram_tensor("wk_aug", [FC + 1, CR], BF16, kind="ExternalInput")
    wa_aug = nc.dram_tensor("wa_aug", [FC + 1, 65], BF16, kind="ExternalInput")
    wox_aug = nc.dram_tensor("wox_aug", [C + 1, C], F32R, kind="ExternalInput")
    woa_t = nc.dram_tensor("woa_t", [C, C], F32R, kind="ExternalInput")
    out_d = nc.dram_tensor("out_c", [C, QPC], F32, kind="ExternalOutput")

    with tile.TileContext(nc) as tc, ExitStack() as ctx:
        consts = ctx.enter_context(tc.tile_pool(name="consts", bufs=1))
        stp = ctx.enter_context(tc.tile_pool(name="stp", bufs=3))
        wk_pool = ctx.enter_context(tc.tile_pool(name="work", bufs=3))
        # PSUM: psAB 2x2 banks (score groups 0,1) + psC 2x1 (score group 2,
        # also warm-up / k,q conv / rb broadcast) + psO 2x1 (acc / fin /
        # v-conv) = 8 banks
        psAB = ctx.enter_context(tc.tile_pool(name="psAB", bufs=2, space="PSUM"))
        psC = ctx.enter_context(tc.tile_pool(name="psC", bufs=2, space="PSUM"))
        psO = ctx.enter_context(tc.tile_pool(name="psO", bufs=2, space="PSUM"))

        # ---- constants / inputs in SBUF ----
        NQT = 4
        HWQ = HW // NQT  # 2304 keys per quarter
        FMq = [
            consts.tile([FC + 1, HWQ], BF16, tag=f"fm{p}", name=f"FM{p}")
            for p in range(NQT)
        ]
        XA = consts.tile([C + 1, QPC], F32R)         # x chunk + ones row
        XB = consts.tile([C + 1, QPC], BF16)         # bf16 copy for q-conv
        WQ = consts.tile([C + 1, CR], BF16)
        WK = consts.tile([FC + 1, CR], BF16)
        WA = consts.tile([FC + 1, 65], BF16)
        WOX = consts.tile([C + 1, C], F32R)
        WOA = consts.tile([C, C], F32R)

        ones1 = consts.tile([1, C], F32R)
        nc.gpsimd.memset(ones1[:].bitcast(F32), 1.0)
        warm_rhs = consts.tile([1, 512], F32R)
        nc.gpsimd.memset(warm_rhs[:].bitcast(F32), 1.0)
        junk_w = consts.tile([128, 512], BF16)  # junk-matmul operands
        nc.gpsimd.memset(junk_w[:], 0.0)
        expb = consts.tile([128, 1], F32)   # exp bias (-SHIFT) for ScalarE
        nc.gpsimd.memset(expb[:], -SHIFT)

        # DMA spread over the 3 dma-capable engine queues; per-queue order =
        # critical path: k-conv q0 needs WK + FM0, q-conv chunk 0 needs WQ +
        # XB[:, :512].
        nc.scalar.dma_start(WK[:], wk_aug.ap())
        nc.scalar.dma_start(WQ[:], wq_aug.ap())
        nc.scalar.dma_start(XB[:, 0:512], x_bf.ap()[:, 0:512])
        nc.scalar.dma_start(XB[:, 512:QPC], x_bf.ap()[:, 512:QPC])
        nc.sync.dma_start(FMq[0][:, 0:768], fm_aug.ap()[:, 0:768])
        nc.sync.dma_start(FMq[0][:, 768:1536], fm_aug.ap()[:, 768:1536])
        nc.sync.dma_start(FMq[0][:, 1536:HWQ], fm_aug.ap()[:, 1536:HWQ])
        nc.sync.dma_start(FMq[1][:], fm_aug.ap()[:, HWQ : 2 * HWQ])
        nc.gpsimd.dma_start(WA[:], wa_aug.ap())
        nc.gpsimd.dma_start(FMq[2][:], fm_aug.ap()[:, 2 * HWQ : 3 * HWQ])
        nc.gpsimd.dma_start(FMq[3][:], fm_aug.ap()[:, 3 * HWQ : 4 * HWQ])
        nc.gpsimd.dma_start(WOX[:], wox_aug.ap())
        nc.gpsimd.dma_start(WOA[:], woa_t.ap())
        nc.gpsimd.dma_start(XA[:], x_aug.ap())

        def fm_kt(kt):  # [65, 128] slice of fusionmap for key tile kt
            p, i = divmod(kt, 18)
            return FMq[p][:, 128 * i : 128 * (i + 1)]

        # KR: k channels row-packed: partitions 32g..32g+31 hold key tile
        # kt=3j+g at free block j; per-quarter for earlier start.
        # QR: per-chunk tiles, q replicated on partition groups 0..2.
        KRq = [
            consts.tile([128, NJ // NQT, 128], BF16, tag=f"kr{p}", name=f"KR{p}")
            for p in range(NQT)
        ]
        QRc = [
            consts.tile([128, qn], BF16, tag=f"qr{ci}", name=f"QR{ci}")
            for ci, (q0, qn) in enumerate(QCHUNKS)
        ]
        # VT: [keys(128), kt-slot(4, stride 80), 80]; cols 0:64 = relu'd v in
        # fp8e4, col 64 = 1.0 (denominator), cols 65:80 never written/read.
        NVR = NKT // 4
        VTr = [
            consts.tile([128, 4, 80], F8, tag=f"vt{r}", name=f"VT{r}")
            for r in range(NVR)
        ]

        # Junk matmuls keep the PE continuously busy from engine-init until
        # the attention loop is self-sustaining.  The HAM clock gate has
        # hysteresis: a 3.4us idle window (e.g. a DMA wait) re-throttles to
        # half clock and the loop's small per-step gaps then never
        # un-throttle it -- the whole kernel runs 2x slow (v1 bimodality,
        # v2 observation: K=4/8 for the first 156us).  Bridging every
        # startup gap with junk makes warm entry deterministic.
        # 128-row contraction: a 1-row junk matmul does NOT register as
        # PE-busy for the HAM (observed: 8us of continuous 1-row matmuls
        # left K at 4/8).  Alternate between two ring tiles so consecutive
        # junk MMs don't WAW-serialize on one PSUM bank.
        def junk(n):
            jp1 = psC.tile([128, 512], F32, tag="c", name="junk1")
            jp2 = psC.tile([128, 512], F32, tag="c", name="junk2")
            for i in range(n):
                nc.tensor.matmul(
                    (jp1 if i % 2 else jp2)[:, :],
                    junk_w[:, 0:128],
                    junk_w[:, 0:512],
                )

        # ---- phase 1: q / k convs ----
        # Column-tiled on the PE: three 32-wide groups (tile_position col_grp)
        # run concurrently and write the row-packed KR layout directly.
        relu_eng = [0]

        def relu(dst, src):
            # alternate conv relus between DVE and ScalarE to keep both
            # engines' exp budgets intact during chunk 0
            relu_eng[0] ^= 1
            if relu_eng[0]:
                nc.vector.tensor_scalar_max(dst, src, 0.0)
            else:
                nc.scalar.activation(dst, src, mybir.ActivationFunctionType.Relu)

        def k_quarter(p, j0, jn):
            fmv = FMq[p].rearrange("p (j g c) -> p j g c", g=G, c=128)
            ps = psC.tile([128, 512], F32, tag="c", name="kps")
            for g in range(G):
                nc.tensor.matmul(
                    ps[32 * g : 32 * g + 32, 0 : jn * 128],
                    WK[:],
                    fmv[:, j0 : j0 + jn, g, :],
                    tile_position=(0, 32 * g),
                )
            relu(KRq[p][0:96, j0 : j0 + jn, :], ps[0:96, 0 : jn * 128])

        def q_chunk(ci):
            q0, qn = QCHUNKS[ci]
            ps = psC.tile([128, 512], F32, tag="c", name="qps")
            for g in range(G):
                nc.tensor.matmul(
                    ps[32 * g : 32 * g + 32, 0:qn],
                    WQ[:],
                    XB[:, q0 : q0 + qn],
                    tile_position=(0, 32 * g),
                )
            relu(QRc[ci][0:96, 0:qn], ps[0:96, 0:qn])

        # prologue: junk bridges every DMA-wait so the PE never sees a 3.4us
        # idle window; conv pieces are emitted as their inputs land.
        junk(20)
        k_quarter(0, 0, 2)
        # exp-table preload (~1.4us scalar-queue) -- after the input DMA
        # descriptors, before the first real exp
        dummy = wk_pool.tile([1, 1], F32, tag="dummy", name="dummy")
        nc.scalar.activation(dummy[:], ones1[0:1, 0:1].bitcast(F32), EXPF)
        junk(4)
        k_quarter(0, 2, 2)
        junk(4)
        q_chunk(0)
        junk(4)
        k_quarter(0, 4, 2)
        junk(4)

        # v^T conv round r: keys 512r..512r+511 -> VTr[r][:, i, 0:65]
        def v_round(r):
            ps = psO.tile([128, 4, 80], F32, tag="acc", name="vps")
            for i in range(4):
                kt = 4 * r + i
                nc.tensor.matmul(ps[:, i, 0:65], fm_kt(kt), WA[:])
            relu(VTr[r][:, :, 0:65], ps[:, :, 0:65])

        # ---- phase 2: attention + output conv, per query chunk ----
        def finalize_a(acc, q0, qn):
            # row C of acc = sum_k exp(score).  Spread the [1, qn] row over
            # 64 partitions by DMA so the DVE reciprocal runs on parallel
            # lanes, gather back.
            Wd = qn // 64
            DS = wk_pool.tile([1, 512], F32, tag="ds", name="ds")
            nc.scalar.copy(DS[:, 0:qn], acc[C : C + 1, 0:qn])
            DD = wk_pool.tile([64, 8], F32, tag="dd", name="dd")
            nc.scalar.dma_start(DD[:, 0:Wd], DS[:, 0:qn])
            RR = wk_pool.tile([64, 8], F32, tag="rr", name="rr")
            with nc.allow_low_precision(reason="softmax denom reciprocal"):
                nc.vector.reciprocal(RR[:, 0:Wd], DD[:, 0:Wd])
            RCP = wk_pool.tile([1, 512], F32R, tag="rc", name="rc")
            nc.scalar.dma_start(RCP[:, 0:qn].bitcast(F32), RR[:, 0:Wd])
            return RCP

        def finalize_b(acc, q0, qn, RCP):
            # Broadcast 1/denom to 64 partitions with a K=1 matmul.
            rb_ps = psC.tile([128, 512], F32, tag="c", name="rb_ps")
            nc.tensor.matmul(rb_ps[0:C, 0:qn], ones1[:], RCP[:, 0:qn])
            rbS = wk_pool.tile([C, 512], F32, tag="rbS", name="rbS")
            nc.scalar.copy(rbS[:, 0:qn], rb_ps[0:C, 0:qn])
            return rbS

        def finalize_c(acc, q0, qn, rbS):
            attn = wk_pool.tile([C, 512], F32R, tag="attn", name="attn")
            nc.vector.tensor_mul(attn[:, 0:qn], acc[0:C, 0:qn], rbS[:, 0:qn])
            fin = psO.tile([128, 512], F32, tag="acc", name="fin")
            nc.tensor.matmul(
                fin[0:C, 0:qn], WOX[:], XA[:, q0 : q0 + qn],
                start=True, stop=False,
            )
            nc.tensor.matmul(
                fin[0:C, 0:qn], WOA[:], attn[:, 0:qn],
                start=False, stop=True,
            )
            outs = wk_pool.tile([C, 512], F32, tag="outs", name="outs")
            nc.scalar.activation(
                outs[:, 0:qn], fin[0:C, 0:qn],
                mybir.ActivationFunctionType.Relu,
            )
            nc.sync.dma_start(out_d.ap()[:, q0 : q0 + qn], outs[:, 0:qn])

        # Flat software-pipelined emission over macro-steps.  A macro-step is
        # one trio of key tiles: one j-group for 512-wide chunks, TWO
        # j-groups side by side for the 256 chunk.  exp output accumulates in
        # a per-PAIR super-tile ST [128, 6, qn] (plane = key tile mod 6) so
        # DoubleRow mm2 reads adjacent planes.
        macro = []  # (qi, subs)
        for qi, (q0, qn) in enumerate(QCHUNKS):
            if qn == 512:
                macro += [(qi, [j]) for j in range(NJ)]
            else:
                macro += [(qi, [2 * u, 2 * u + 1]) for u in range(NJ // 2)]
        accs = [None] * len(QCHUNKS)
        pend_pairs = []   # (qi, u, st_tile)
        PAIR_LAG = 1
        pend_fin = None
        pend_rcp = None
        vr_next = 0

        def emit_pair_group(qi, u, st):
            q0, qn = QCHUNKS[qi]
            for h in range(3):
                kt0 = 6 * u + 2 * h
                r, i = divmod(kt0, 4)
                nc.tensor.matmul(
                    accs[qi][0:65, 0:qn],
                    VTr[r][:, i : i + 2, 0:65],
                    st[:, 2 * h : 2 * h + 2, 0:qn],
                    start=(kt0 == 0),
                    stop=(kt0 == NKT - 2),
                    perf_mode=DR,
                )

        st_cur = None
        for t, (qi, subs) in enumerate(macro):
            q0, qn = QCHUNKS[qi]
            j = subs[0]
            if j == 0:
                accs[qi] = psO.tile([128, 512], F32, tag="acc", name="acc")
                mi = 0
            mi += 1
            if qi == 0:
                KSCHED = {
                    0: lambda: k_quarter(1, 0, 4),
                    1: lambda: k_quarter(1, 4, 2),
                    3: lambda: k_quarter(2, 0, 4),
                    4: lambda: k_quarter(2, 4, 2),
                    6: lambda: q_chunk(1),
                    8: lambda: k_quarter(3, 0, 4),
                    9: lambda: k_quarter(3, 4, 2),
                }
                if j in KSCHED:
                    KSCHED[j]()
            elif qi < len(QCHUNKS) - 1 and j == 4:
                q_chunk(qi + 1)
            while vr_next < NVR and (qi > 0 or 4 * vr_next <= 3 * j + 10):
                v_round(vr_next)
                vr_next += 1

            abt = psAB.tile([128, 2, 512], F32, tag="ab", name="abt")
            ct = psC.tile([128, 512], F32, tag="c", name="ct")
            # mm1: groups 0,1 -> abt banks, group 2 -> ct
            for h, sj in enumerate(subs):
                jq, jj = divmod(sj, NJ // NQT)
                for g in range(G):
                    tgt = (
                        abt[:, g, qn * h : qn * h + qn]
                        if g < 2
                        else ct[:, qn * h : qn * h + qn]
                    )
                    nc.tensor.matmul(
                        tgt,
                        KRq[jq][32 * g : 32 * g + 32, jj, :],
                        QRc[qi][32 * g : 32 * g + 32, 0:qn],
                    )
            # idempotent PE filler: re-emit the g=0 matmul of this step.
            # Raises PE duty from ~75% to ~87% so the HAM clock gate stays
            # at full rate through the exp-bound steady state; at full
            # clock it mostly fills genuine idle.
            sj0 = subs[0]
            jq0, jj0 = divmod(sj0, NJ // NQT)
            nc.tensor.matmul(
                abt[:, 0, 0:qn],
                KRq[jq0][0:32, jj0, :],
                QRc[qi][0:32, 0:qn],
            )
            # exp: both engines concurrently; ScalarE (faster) takes the
            # 2-slot psAB tile on 2 of 3 steps; chunk 0 alternates 50/50
            # because the conv relus also land on these engines.
            scalar_ab = (t % 2 == 0) if qi == 0 else (t % 3 != 2)
            if qn == 512:
                if j % 2 == 0:
                    st_cur = stp.tile([128, 6, 512], F8, tag="st", name="st")
                base = 3 * (j % 2)
                ab_dst = st_cur[:, base : base + 2, :]
                c_dst = st_cur[:, base + 2, :]
                if scalar_ab:
                    nc.scalar.activation(ab_dst, abt[:], EXPF, bias=expb[:, :])
                    nc.vector.tensor_scalar(
                        c_dst.bitcast(I8), ct[:, 0:qn], S8, B8C,
                        mybir.AluOpType.mult, mybir.AluOpType.add,
                    )
                else:
                    nc.vector.tensor_scalar(
                        ab_dst.bitcast(I8), abt[:], S8, B8C,
                        mybir.AluOpType.mult, mybir.AluOpType.add,
                    )
                    nc.scalar.activation(c_dst, ct[:, 0:qn], EXPF, bias=expb[:, :])
                if j % 2 == 1:
                    pend_pairs.append((qi, j // 2, st_cur))
            else:
                # 256 chunk: one step covers 6 key tiles (two j-groups over
                # the same 256 queries).  Plane remap: kt 6u+3h+g <- group g,
                # column half h.
                st_cur = stp.tile([128, 6, 256], F8, tag="st256", name="st256")
                for h in range(2):
                    ab_dst = st_cur[:, 3 * h : 3 * h + 2, :]
                    ab_src = abt[:, :, 256 * h : 256 * h + 256]
                    c_dst = st_cur[:, 3 * h + 2, :]
                    c_src = ct[:, 256 * h : 256 * h + 256]
                    if scalar_ab:
                        nc.scalar.activation(ab_dst, ab_src, EXPF, bias=expb[:, :])
                        nc.vector.tensor_scalar(
                            c_dst.bitcast(I8), c_src, S8, B8C,
                            mybir.AluOpType.mult, mybir.AluOpType.add,
                        )
                    else:
                        nc.vector.tensor_scalar(
                            ab_dst.bitcast(I8), ab_src, S8, B8C,
                            mybir.AluOpType.mult, mybir.AluOpType.add,
                        )
                        nc.scalar.activation(c_dst, c_src, EXPF, bias=expb[:, :])
                pend_pairs.append((qi, subs[0] // 2, st_cur))
            while len(pend_pairs) > PAIR_LAG:
                emit_pair_group(*pend_pairs.pop(0))
            if pend_fin is not None:
                if mi == 4:
                    pend_rcp = finalize_a(*pend_fin)
                elif mi == 6:
                    pend_rcp = finalize_b(*pend_fin, pend_rcp)
                elif mi == 7:
                    finalize_c(*pend_fin, pend_rcp)
                    pend_fin = None
                    pend_rcp = None
            if subs[-1] == NJ - 1:
                pend_fin = (accs[qi], q0, qn)
        # tail: flush remaining mm2, then a short finalize -- the reciprocal
        # runs directly on the [1, qn] denominator row (no DMA round trips;
        # nothing else overlaps the tail so latency is all that matters).
        for p in pend_pairs:
            emit_pair_group(*p)
        acc, q0, qn = pend_fin
        fin = psO.tile([128, 512], F32, tag="acc", name="fin")
        nc.tensor.matmul(
            fin[0:C, 0:qn], WOX[:], XA[:, q0 : q0 + qn],
            start=True, stop=False,
        )
        RCP = wk_pool.tile([1, 512], F32R, tag="rc", name="rc2")
        with nc.allow_low_precision(reason="softmax denom reciprocal"):
            nc.vector.reciprocal(RCP[:, 0:qn], acc[C : C + 1, 0:qn])
        rbs = finalize_b(acc, q0, qn, RCP)
        attn = wk_pool.tile([C, 512], F32R, tag="attn", name="attn")
        nc.vector.tensor_mul(attn[:, 0:qn], acc[0:C, 0:qn], rbs[:, 0:qn])
        nc.tensor.matmul(
            fin[0:C, 0:qn], WOA[:], attn[:, 0:qn],
            start=False, stop=True,
        )
        outs = wk_pool.tile([C, 512], F32, tag="outs", name="outs")
        nc.vector.tensor_scalar_max(outs[:, 0:qn], fin[0:C, 0:qn], 0.0)
        nc.sync.dma_start(out_d.ap()[:, q0 : q0 + qn], outs[:, 0:qn])

    nc.compile()
    return nc


_NC_CACHE = None


def _get_nc():
    global _NC_CACHE
    if _NC_CACHE is None:
        _NC_CACHE = build_bass()
    return _NC_CACHE


def make_in_maps(x, fusionmap, wq, bq, wk, bk, wa, ba, wo, bo):
    x = np.asarray(x, np.float32)
    fm = np.asarray(fusionmap, np.float32)
    xf = x.reshape(N, C, HW)
    fmf = fm.reshape(N, FC, HW)
    ones_hw = np.ones((1, HW), np.float32)
    wq_aug = np.concatenate(
        [np.asarray(wq).T, np.asarray(bq)[None, :]], 0
    ).astype(ml_dtypes.bfloat16)
    wk_aug = np.concatenate(
        [np.asarray(wk).T, np.asarray(bk)[None, :]], 0
    ).astype(ml_dtypes.bfloat16)
    # [wa^T | 0; ba | 1]: column C evaluates to exactly 1.0 after the conv
    # (ones row of fm_aug x ones col), giving mm2 its denominator row.
    wa_blk = np.concatenate([np.asarray(wa).T, np.asarray(ba)[None, :]], 0)
    ones_blk = np.concatenate(
        [np.zeros((FC, 1), np.float32), np.ones((1, 1), np.float32)], 0
    )
    wa_aug = np.concatenate([wa_blk, ones_blk], 1).astype(ml_dtypes.bfloat16)
    wo = np.asarray(wo, np.float32)
    wox_aug = np.concatenate(
        [wo[:, :C].T, np.asarray(bo)[None, :]], 0
    ).astype(np.float32)
    woa_t = np.ascontiguousarray(wo[:, C:].T).astype(np.float32)

    in_maps = []
    for core in range(NCORES):
        n, c = divmod(core, 4)
        x_chunk = xf[n][:, c * QPC : (c + 1) * QPC]
        x_aug = np.concatenate([x_chunk, ones_hw[:, :QPC]], 0)
        fm_aug = np.concatenate([fmf[n], ones_hw], 0).astype(ml_dtypes.bfloat16)
        in_maps.append(
            {
                "x_aug": np.ascontiguousarray(x_aug),
                "x_bf": np.ascontiguousarray(x_aug.astype(ml_dtypes.bfloat16)),
                "fm_aug": np.ascontiguousarray(fm_aug),
                "wq_aug": wq_aug,
                "wk_aug": wk_aug,
                "wa_aug": wa_aug,
                "wox_aug": wox_aug,
                "woa_t": woa_t,
            }
        )
    return in_maps


def run(in_maps, trace=False, tmpdir=None):
    nc = _get_nc()
    return bass_utils.run_bass_kernel_spmd(
        nc,
        in_maps,
        core_ids=list(range(NCORES)),
        trace=trace,
        tmpdir=tmpdir,
    )


def kernel(**inputs):
    in_maps = make_in_maps(**inputs)
    res = run(in_maps)
    out = np.empty((N, C, HW), np.float32)
    for core in range(NCORES):
        n, c = divmod(core, 4)
        out[n][:, c * QPC : (c + 1) * QPC] = res.results[core]["out_c"]
    return out.reshape(N, C, H, W)


if __name__ == "__main__":
    import reference

    inputs = {k: np.asarray(v) for k, v in reference.setup_inputs().items()}
    got = kernel(**inputs)
    print("kernel output", got.shape, got.dtype)


# revision 15
# speedup vs baseline: 1.1829x; 1.1829x over previous
"""Trainium2 Bass kernel for NonLocalAttention (fused 1x1 convs + spatial softmax attention).

Reference computation (N=2, C=64, FC=64, CR=32, H=W=96, HW=9216):
    q = relu(wq @ x + bq)          [N, 32, HW]
    k = relu(wk @ fm + bk)         [N, 32, HW]
    v = relu(wa @ fm + ba)         [N, 64, HW]
    s = softmax(q^T k, axis=keys)  [N, HW, HW]
    o = s @ v^T                    [N, HW, 64]
    out = relu(wo @ [x; o^T] + bo) [N, 64, HW]

Sharding: 8 cores = batch(2) x query-rows(4).  Each core handles 2304 query
pixels of one batch element and needs the full fusionmap of that batch.

Per-core kernel (flash-style, score never goes to HBM), v2 = fp8 DoubleRow:
  - score computed TRANSPOSED: st[key, q] = k^T q via row-packed (K=32)
    matmuls, 3 key tiles at a time (G=3): groups 0,1 land in a 2-bank PSUM
    tile (psAB), group 2 in a 1-bank tile (psC).  Separate tiles so the two
    exp engines each read their OWN tile (the tile scheduler serializes
    cross-engine readers of a shared tile -- bank splits of one tile ran
    lockstep in v1).
  - exp runs CONCURRENTLY on both engines every step: one engine takes psAB
    (1024 elems/partition), the other psC (512).  ScalarE does exact LUT
    exp(x-SHIFT) -> fp8e4; VectorE does an int8 Schraudolph (y=int8(x*S8+B8);
    the int8 bit pattern read as e4m3 is ~exp(x-SHIFT), +-8% sawtooth that
    softmax normalization cancels -- measured end-to-end rel err 1.8e-4).
    ScalarE is the faster engine (1.2 vs ~1.0 elem/ns/partition) so it takes
    psAB on 2 of 3 steps.  SHIFT=1.6 keeps exp in [0.2, 235]: TRN fp8e4
    tops out at 240 (248+ = inf), and softmax is shift-invariant.
  - exp output goes to a 2-step ST super-tile [128, 6, 512] so consecutive
    key tiles sit in adjacent planes -> mm2 runs fp8 DoubleRow: each matmul
    contracts a PAIR of key tiles (256 keys) streaming 2 elems/cycle, half
    the v1 mm2 stream time.  lhsT = VT pairs [128, 2, 65]; column 64 is 1.0
    straight out of the v-conv (wa augmented with a ones column) so PSUM row
    64 accumulates the softmax denominator for free.
  - v-conv writes only cols 0:65 of each [128, 80]-strided VT slot (v1
    relu'd 128 wide; cols 65+ are never read -- garbage is confined to acc
    rows 65..127 which are never read either).
  - mm2 pair-groups lag ~2 macro-steps behind mm1 so the in-order PE queue
    never sits behind an unfinished exp; mm1 pairs are emitted back-to-back
    (halves mm1<->mm2 stationary transitions).
  - ~7us junk-matmul warm-up during the initial DMA wait locks the HAM
    clock gate at full rate (without it runs enter the loop at half PE
    clock).
  - input DMAs are spread over four engine queues (sync/scalar/vector/
    gpsimd) so descriptor generation is not serialized on one engine
    (v1: first matmul at ~16us; the DMA bytes themselves are ~2.3MB).
  - normalize: PSUM row 64 -> spread over 64 partitions by DMA, DVE
    reciprocal, gather back, broadcast by a K=1 matmul.  The LAST chunk
    instead runs the reciprocal directly on the [1, 512] row (DVE streams
    it through one lane, ~540ns) -- latency beats the two DMA round trips
    when nothing overlaps the tail.
"""

import sys

sys.path.insert(0, "/opt/trn_rl_repo")

from contextlib import ExitStack

import ml_dtypes
import numpy as np

import concourse.bacc as bacc
import concourse.bass as bass
import concourse.tile as tile
from concourse import mybir
from concourse import bass_utils

C = 64
FC = 64
CR = 32
N = 2
H = W = 96
HW = H * W            # 9216
NCORES = 8
QPC = HW // 4         # queries per core = 2304
NKT = HW // 128       # 72 key tiles
G = 3                 # row-packing group (3 key tiles concurrently)
NJ = NKT // G         # 24 key-tile groups
# the 256 chunk runs LAST: its lighter PE load can drop the HAM clock gate
# to half rate, which is fatal mid-kernel (once cold, the loop never
# un-throttles) but costs only ~2us right before the tail.
QCHUNKS = [(0, 512), (512, 512), (1024, 512), (1536, 512), (2048, 256)]

F32 = mybir.dt.float32
F32R = mybir.dt.float32r
BF16 = mybir.dt.bfloat16
F8 = mybir.dt.float8e4
I8 = mybir.dt.int8
DR = mybir.MatmulPerfMode.DoubleRow
EXPF = mybir.ActivationFunctionType.Exp

LOG2E = 1.4426950408889634
SHIFT = 1.6           # exp(s-SHIFT): s<=~6.5 -> exp<=~235 < fp8e4 max 240
S8 = 8.0 * LOG2E
# int8 Schraudolph bias: (7<<3) - 8*0.0579 (sawtooth centering), minus the
# folded shift.
B8C = 55.537 - SHIFT * S8


def build_bass():
    nc = bacc.Bacc(
        "TRN2", target_bir_lowering=False, debug=False, num_devices=NCORES
    )

    x_aug = nc.dram_tensor("x_aug", [C + 1, QPC], F32R, kind="ExternalInput")
    x_bf = nc.dram_tensor("x_bf", [C + 1, QPC], BF16, kind="ExternalInput")
    fm_aug = nc.dram_tensor("fm_aug", [FC + 1, HW], BF16, kind="ExternalInput")
    wq_aug = nc.dram_tensor("wq_aug", [C + 1, CR], BF16, kind="ExternalInput")
    wk_aug = nc.dram_tensor("wk_aug", [FC + 1, CR], BF16, kind="ExternalInput")
    wa_aug = nc.dram_tensor("wa_aug", [FC + 1, 65], BF16, kind="ExternalInput")
    wox_aug = nc.dram_tensor("wox_aug", [C + 1, C], F32R, kind="ExternalInput")
    woa_t = nc.dram_tensor("woa_t", [C, C], F32R, kind="ExternalInput")
    out_d = nc.dram_tensor("out_c", [C, QPC], F32, kind="ExternalOutput")

    with tile.TileContext(nc) as tc, ExitStack() as ctx:
        consts = ctx.enter_context(tc.tile_pool(name="consts", bufs=1))
        stp = ctx.enter_context(tc.tile_pool(name="stp", bufs=3))
        wk_pool = ctx.enter_context(tc.tile_pool(name="work", bufs=3))
        # PSUM: psAB 2x2 banks (score groups 0,1) + psC 2x1 (score group 2,
        # also warm-up / k,q conv / rb broadcast) + psO 2x1 (acc / fin /
        # v-conv) = 8 banks
        psAB = ctx.enter_context(tc.tile_pool(name="psAB", bufs=2, space="PSUM"))
        psC = ctx.enter_context(tc.tile_pool(name="psC", bufs=2, space="PSUM"))
        psO = ctx.enter_context(tc.tile_pool(name="psO", bufs=2, space="PSUM"))

        # ---- constants / inputs in SBUF ----
        NQT = 4
        HWQ = HW // NQT  # 2304 keys per quarter
        FMq = [
            consts.tile([FC + 1, HWQ], BF16, tag=f"fm{p}", name=f"FM{p}")
            for p in range(NQT)
        ]
        XA = consts.tile([C + 1, QPC], F32R)         # x chunk + ones row
        XB = consts.tile([C + 1, QPC], BF16)         # bf16 copy for q-conv
        WQ = consts.tile([C + 1, CR], BF16)
        WK = consts.tile([FC + 1, CR], BF16)
        WA = consts.tile([FC + 1, 65], BF16)
        WOX = consts.tile([C + 1, C], F32R)
        WOA = consts.tile([C, C], F32R)

        ones1 = consts.tile([1, C], F32R)
        nc.gpsimd.memset(ones1[:].bitcast(F32), 1.0)
        warm_rhs = consts.tile([1, 512], F32R)
        nc.gpsimd.memset(warm_rhs[:].bitcast(F32), 1.0)
        junk_w = consts.tile([128, 512], BF16)  # junk-matmul operands
        nc.gpsimd.memset(junk_w[:], 0.0)
        expb = consts.tile([128, 1], F32)   # exp bias (-SHIFT) for ScalarE
        nc.gpsimd.memset(expb[:], -SHIFT)

        # DMA spread over the 3 dma-capable engine queues; per-queue order =
        # critical path: k-conv q0 needs WK + FM0, q-conv chunk 0 needs WQ +
        # XB[:, :512].
        nc.scalar.dma_start(WK[:], wk_aug.ap())
        nc.scalar.dma_start(WQ[:], wq_aug.ap())
        nc.scalar.dma_start(XB[:, 0:512], x_bf.ap()[:, 0:512])
        nc.scalar.dma_start(XB[:, 512:QPC], x_bf.ap()[:, 512:QPC])
        nc.sync.dma_start(FMq[0][:, 0:768], fm_aug.ap()[:, 0:768])
        nc.sync.dma_start(FMq[0][:, 768:1536], fm_aug.ap()[:, 768:1536])
        nc.sync.dma_start(FMq[0][:, 1536:HWQ], fm_aug.ap()[:, 1536:HWQ])
        nc.sync.dma_start(FMq[1][:], fm_aug.ap()[:, HWQ : 2 * HWQ])
        nc.gpsimd.dma_start(WA[:], wa_aug.ap())
        nc.gpsimd.dma_start(FMq[2][:], fm_aug.ap()[:, 2 * HWQ : 3 * HWQ])
        nc.gpsimd.dma_start(FMq[3][:], fm_aug.ap()[:, 3 * HWQ : 4 * HWQ])
        nc.gpsimd.dma_start(WOX[:], wox_aug.ap())
        nc.gpsimd.dma_start(WOA[:], woa_t.ap())
        nc.gpsimd.dma_start(XA[:], x_aug.ap())

        def fm_kt(kt):  # [65, 128] slice of fusionmap for key tile kt
            p, i = divmod(kt, 18)
            return FMq[p][:, 128 * i : 128 * (i + 1)]

        # KR: k channels row-packed: partitions 32g..32g+31 hold key tile
        # kt=3j+g at free block j; per-quarter for earlier start.
        # QR: per-chunk tiles, q replicated on partition groups 0..2.
        KRq = [
            consts.tile([128, NJ // NQT, 128], BF16, tag=f"kr{p}", name=f"KR{p}")
            for p in range(NQT)
        ]
        QRc = [
            consts.tile([128, qn], BF16, tag=f"qr{ci}", name=f"QR{ci}")
            for ci, (q0, qn) in enumerate(QCHUNKS)
        ]
        # VT: [keys(128), kt-slot(4, stride 80), 80]; cols 0:64 = relu'd v in
        # fp8e4, col 64 = 1.0 (denominator), cols 65:80 never written/read.
        NVR = NKT // 4
        VTr = [
            consts.tile([128, 4, 80], F8, tag=f"vt{r}", name=f"VT{r}")
            for r in range(NVR)
        ]

        # Junk matmuls keep the PE continuously busy from engine-init until
        # the attention loop is self-sustaining.  The HAM clock gate has
        # hysteresis: a 3.4us idle window (e.g. a DMA wait) re-throttles to
        # half clock and the loop's small per-step gaps then never
        # un-throttle it -- the whole kernel runs 2x slow (v1 bimodality,
        # v2 observation: K=4/8 for the first 156us).  Bridging every
        # startup gap with junk makes warm entry deterministic.
        # 128-row contraction: a 1-row junk matmul does NOT register as
        # PE-busy for the HAM (observed: 8us of continuous 1-row matmuls
        # left K at 4/8).  Alternate between two ring tiles so consecutive
        # junk MMs don't WAW-serialize on one PSUM bank.
        def junk(n):
            jp1 = psC.tile([128, 512], F32, tag="c", name="junk1")
            jp2 = psC.tile([128, 512], F32, tag="c", name="junk2")
            for i in range(n):
                nc.tensor.matmul(
                    (jp1 if i % 2 else jp2)[:, :],
                    junk_w[:, 0:128],
                    junk_w[:, 0:512],
                )

        # ---- phase 1: q / k convs ----
        # Column-tiled on the PE: three 32-wide groups (tile_position col_grp)
        # run concurrently and write the row-packed KR layout directly.
        relu_eng = [0]

        def relu(dst, src):
            # alternate conv relus between DVE and ScalarE to keep both
            # engines' exp budgets intact during chunk 0
            relu_eng[0] ^= 1
            if relu_eng[0]:
                nc.vector.tensor_scalar_max(dst, src, 0.0)
            else:
                nc.scalar.activation(dst, src, mybir.ActivationFunctionType.Relu)

        def k_quarter(p, j0, jn):
            fmv = FMq[p].rearrange("p (j g c) -> p j g c", g=G, c=128)
            ps = psC.tile([128, 512], F32, tag="c", name="kps")
            for g in range(G):
                nc.tensor.matmul(
                    ps[32 * g : 32 * g + 32, 0 : jn * 128],
                    WK[:],
                    fmv[:, j0 : j0 + jn, g, :],
                    tile_position=(0, 32 * g),
                )
            relu(KRq[p][0:96, j0 : j0 + jn, :], ps[0:96, 0 : jn * 128])

        def q_chunk(ci):
            q0, qn = QCHUNKS[ci]
            ps = psC.tile([128, 512], F32, tag="c", name="qps")
            for g in range(G):
                nc.tensor.matmul(
                    ps[32 * g : 32 * g + 32, 0:qn],
                    WQ[:],
                    XB[:, q0 : q0 + qn],
                    tile_position=(0, 32 * g),
                )
            relu(QRc[ci][0:96, 0:qn], ps[0:96, 0:qn])

        # prologue: junk bridges every DMA-wait so the PE never sees a 3.4us
        # idle window; conv pieces are emitted as their inputs land.
        junk(20)
        k_quarter(0, 0, 2)
        # exp-table preload (~1.4us scalar-queue) -- after the input DMA
        # descriptors, before the first real exp
        dummy = wk_pool.tile([1, 1], F32, tag="dummy", name="dummy")
        nc.scalar.activation(dummy[:], ones1[0:1, 0:1].bitcast(F32), EXPF)
        junk(4)
        k_quarter(0, 2, 2)
        junk(4)
        q_chunk(0)
        junk(4)
        k_quarter(0, 4, 2)
        junk(4)

        # v^T conv round r: keys 512r..512r+511 -> VTr[r][:, i, 0:65]
        def v_round(r):
            ps = psO.tile([128, 4, 80], F32, tag="acc", name="vps")
            for i in range(4):
                kt = 4 * r + i
                nc.tensor.matmul(ps[:, i, 0:65], fm_kt(kt), WA[:])
            relu(VTr[r][:, :, 0:65], ps[:, :, 0:65])

        # ---- phase 2: attention + output conv, per query chunk ----
        def finalize_a(acc, q0, qn):
            # row C of acc = sum_k exp(score).  Spread the [1, qn] row over
            # 64 partitions by DMA so the DVE reciprocal runs on parallel
            # lanes, gather back.
            Wd = qn // 64
            DS = wk_pool.tile([1, 512], F32, tag="ds", name="ds")
            nc.scalar.copy(DS[:, 0:qn], acc[C : C + 1, 0:qn])
            DD = wk_pool.tile([64, 8], F32, tag="dd", name="dd")
            nc.scalar.dma_start(DD[:, 0:Wd], DS[:, 0:qn])
            RR = wk_pool.tile([64, 8], F32, tag="rr", name="rr")
            with nc.allow_low_precision(reason="softmax denom reciprocal"):
                nc.vector.reciprocal(RR[:, 0:Wd], DD[:, 0:Wd])
            RCP = wk_pool.tile([1, 512], F32R, tag="rc", name="rc")
            nc.scalar.dma_start(RCP[:, 0:qn].bitcast(F32), RR[:, 0:Wd])
            return RCP

        def finalize_b(acc, q0, qn, RCP):
            # Broadcast 1/denom to 64 partitions with a K=1 matmul.
            rb_ps = psC.tile([128, 512], F32, tag="c", name="rb_ps")
            nc.tensor.matmul(rb_ps[0:C, 0:qn], ones1[:], RCP[:, 0:qn])
            rbS = wk_pool.tile([C, 512], F32, tag="rbS", name="rbS")
            nc.scalar.copy(rbS[:, 0:qn], rb_ps[0:C, 0:qn])
            return rbS

        def finalize_c(acc, q0, qn, rbS):
            attn = wk_pool.tile([C, 512], F32R, tag="attn", name="attn")
            nc.vector.tensor_mul(attn[:, 0:qn], acc[0:C, 0:qn], rbS[:, 0:qn])
            fin = psO.tile([128, 512], F32, tag="acc", name="fin")
            nc.tensor.matmul(
                fin[0:C, 0:qn], WOX[:], XA[:, q0 : q0 + qn],
                start=True, stop=False,
            )
            nc.tensor.matmul(
                fin[0:C, 0:qn], WOA[:], attn[:, 0:qn],
                start=False, stop=True,
            )
            outs = wk_pool.tile([C, 512], F32, tag="outs", name="outs")
            nc.scalar.activation(
                outs[:, 0:qn], fin[0:C, 0:qn],
                mybir.ActivationFunctionType.Relu,
            )
            nc.sync.dma_start(out_d.ap()[:, q0 : q0 + qn], outs[:, 0:qn])

        # Flat software-pipelined emission over macro-steps.  A macro-step is
        # one trio of key tiles: one j-group for 512-wide chunks, TWO
        # j-groups side by side for the 256 chunk.  exp output accumulates in
        # a per-PAIR super-tile ST [128, 6, qn] (plane = key tile mod 6) so
        # DoubleRow mm2 reads adjacent planes.
        macro = []  # (qi, subs)
        for qi, (q0, qn) in enumerate(QCHUNKS):
            if qn == 512:
                macro += [(qi, [j]) for j in range(NJ)]
            else:
                macro += [(qi, [2 * u, 2 * u + 1]) for u in range(NJ // 2)]
        accs = [None] * len(QCHUNKS)
        pend_pairs = []   # (qi, u, st_tile)
        PAIR_LAG = 1
        pend_fin = None
        pend_rcp = None
        vr_next = 0

        def emit_pair_group(qi, u, st):
            q0, qn = QCHUNKS[qi]
            for h in range(3):
                kt0 = 6 * u + 2 * h
                r, i = divmod(kt0, 4)
                nc.tensor.matmul(
                    accs[qi][0:65, 0:qn],
                    VTr[r][:, i : i + 2, 0:65],
                    st[:, 2 * h : 2 * h + 2, 0:qn],
                    start=(kt0 == 0),
                    stop=(kt0 == NKT - 2),
                    perf_mode=DR,
                )

        st_cur = None
        for t, (qi, subs) in enumerate(macro):
            q0, qn = QCHUNKS[qi]
            j = subs[0]
            if j == 0:
                accs[qi] = psO.tile([128, 512], F32, tag="acc", name="acc")
                mi = 0
            mi += 1
            if qi == 0:
                KSCHED = {
                    0: lambda: k_quarter(1, 0, 4),
                    1: lambda: k_quarter(1, 4, 2),
                    3: lambda: k_quarter(2, 0, 4),
                    4: lambda: k_quarter(2, 4, 2),
                    6: lambda: q_chunk(1),
                    8: lambda: k_quarter(3, 0, 4),
                    9: lambda: k_quarter(3, 4, 2),
                }
                if j in KSCHED:
                    KSCHED[j]()
            elif qi < len(QCHUNKS) - 1 and j == 4:
                q_chunk(qi + 1)
            while vr_next < NVR and (qi > 0 or 4 * vr_next <= 3 * j + 10):
                v_round(vr_next)
                vr_next += 1

            abt = psAB.tile([128, 2, 512], F32, tag="ab", name="abt")
            ct = psC.tile([128, 512], F32, tag="c", name="ct")
            # mm1: groups 0,1 -> abt banks, group 2 -> ct
            for h, sj in enumerate(subs):
                jq, jj = divmod(sj, NJ // NQT)
                for g in range(G):
                    tgt = (
                        abt[:, g, qn * h : qn * h + qn]
                        if g < 2
                        else ct[:, qn * h : qn * h + qn]
                    )
                    nc.tensor.matmul(
                        tgt,
                        KRq[jq][32 * g : 32 * g + 32, jj, :],
                        QRc[qi][32 * g : 32 * g + 32, 0:qn],
                    )
            # idempotent PE filler: re-emit the g=0 matmul(s) of this step.
            # Raises PE duty from ~75% to ~87% so the HAM clock gate stays
            # at full rate through the exp-bound steady state; at full
            # clock it mostly fills genuine idle.
            for h, sj in enumerate(subs):
                jq0, jj0 = divmod(sj, NJ // NQT)
                nc.tensor.matmul(
                    abt[:, 0, qn * h : qn * h + qn],
                    KRq[jq0][0:32, jj0, :],
                    QRc[qi][0:32, 0:qn],
                )
            # exp: both engines concurrently; ScalarE (faster) takes the
            # 2-slot psAB tile on 2 of 3 steps; chunk 0 alternates 50/50
            # because the conv relus also land on these engines.
            scalar_ab = (t % 2 == 0) if qi == 0 else (t % 3 != 2)
            if qn == 512:
                if j % 2 == 0:
                    st_cur = stp.tile([128, 6, 512], F8, tag="st", name="st")
                base = 3 * (j % 2)
                ab_dst = st_cur[:, base : base + 2, :]
                c_dst = st_cur[:, base + 2, :]
                if scalar_ab:
                    nc.scalar.activation(ab_dst, abt[:], EXPF, bias=expb[:, :])
                    nc.vector.tensor_scalar(
                        c_dst.bitcast(I8), ct[:, 0:qn], S8, B8C,
                        mybir.AluOpType.mult, mybir.AluOpType.add,
                    )
                else:
                    nc.vector.tensor_scalar(
                        ab_dst.bitcast(I8), abt[:], S8, B8C,
                        mybir.AluOpType.mult, mybir.AluOpType.add,
                    )
                    nc.scalar.activation(c_dst, ct[:, 0:qn], EXPF, bias=expb[:, :])
                if j % 2 == 1:
                    pend_pairs.append((qi, j // 2, st_cur))
            else:
                # 256 chunk: one step covers 6 key tiles (two j-groups over
                # the same 256 queries).  Plane remap: kt 6u+3h+g <- group g,
                # column half h.
                st_cur = stp.tile([128, 6, 256], F8, tag="st256", name="st256")
                for h in range(2):
                    ab_dst = st_cur[:, 3 * h : 3 * h + 2, :]
                    ab_src = abt[:, :, 256 * h : 256 * h + 256]
                    c_dst = st_cur[:, 3 * h + 2, :]
                    c_src = ct[:, 256 * h : 256 * h + 256]
                    if scalar_ab:
                        nc.scalar.activation(ab_dst, ab_src, EXPF, bias=expb[:, :])
                        nc.vector.tensor_scalar(
                            c_dst.bitcast(I8), c_src, S8, B8C,
                            mybir.AluOpType.mult, mybir.AluOpType.add,
                        )
                    else:
                        nc.vector.tensor_scalar(
                            ab_dst.bitcast(I8), ab_src, S8, B8C,
                            mybir.AluOpType.mult, mybir.AluOpType.add,
                        )
                        nc.scalar.activation(c_dst, c_src, EXPF, bias=expb[:, :])
                pend_pairs.append((qi, subs[0] // 2, st_cur))
            while len(pend_pairs) > PAIR_LAG:
                emit_pair_group(*pend_pairs.pop(0))
            if pend_fin is not None:
                if mi == 4:
                    pend_rcp = finalize_a(*pend_fin)
                elif mi == 6:
                    pend_rcp = finalize_b(*pend_fin, pend_rcp)
                elif mi == 7:
                    finalize_c(*pend_fin, pend_rcp)
                    pend_fin = None
                    pend_rcp = None
            if subs[-1] == NJ - 1:
                pend_fin = (accs[qi], q0, qn)
        # tail: flush remaining mm2, then a short finalize -- the reciprocal
        # runs directly on the [1, qn] denominator row (no DMA round trips;
        # nothing else overlaps the tail so latency is all that matters).
        for p in pend_pairs:
            emit_pair_group(*p)
        acc, q0, qn = pend_fin
        fin = psO.tile([128, 512], F32, tag="acc", name="fin")
        nc.tensor.matmul(
            fin[0:C, 0:qn], WOX[:], XA[:, q0 : q0 + qn],
            start=True, stop=False,
        )
        RCP = wk_pool.tile([1, 512], F32R, tag="rc", name="rc2")
        with nc.allow_low_precision(reason="softmax denom reciprocal"):
            nc.vector.reciprocal(RCP[:, 0:qn], acc[C : C + 1, 0:qn])
        rbs = finalize_b(acc, q0, qn, RCP)
        attn = wk_pool.tile([C, 512], F32R, tag="attn", name="attn")
        nc.vector.tensor_mul(attn[:, 0:qn], acc[0:C, 0:qn], rbs[:, 0:qn])
        nc.tensor.matmul(
            fin[0:C, 0:qn], WOA[:], attn[:, 0:qn],
            start=False, stop=True,
        )
        outs = wk_pool.tile([C, 512], F32, tag="outs", name="outs")
        nc.vector.tensor_scalar_max(outs[:, 0:qn], fin[0:C, 0:qn], 0.0)
        nc.sync.dma_start(out_d.ap()[:, q0 : q0 + qn], outs[:, 0:qn])

    nc.compile()
    return nc


_NC_CACHE = None


def _get_nc():
    global _NC_CACHE
    if _NC_CACHE is None:
        _NC_CACHE = build_bass()
    return _NC_CACHE


def make_in_maps(x, fusionmap, wq, bq, wk, bk, wa, ba, wo, bo):
    x = np.asarray(x, np.float32)
    fm = np.asarray(fusionmap, np.float32)
    xf = x.reshape(N, C, HW)
    fmf = fm.reshape(N, FC, HW)
    ones_hw = np.ones((1, HW), np.float32)
    wq_aug = np.concatenate(
        [np.asarray(wq).T, np.asarray(bq)[None, :]], 0
    ).astype(ml_dtypes.bfloat16)
    wk_aug = np.concatenate(
        [np.asarray(wk).T, np.asarray(bk)[None, :]], 0
    ).astype(ml_dtypes.bfloat16)
    # [wa^T | 0; ba | 1]: column C evaluates to exactly 1.0 after the conv
    # (ones row of fm_aug x ones col), giving mm2 its denominator row.
    wa_blk = np.concatenate([np.asarray(wa).T, np.asarray(ba)[None, :]], 0)
    ones_blk = np.concatenate(
        [np.zeros((FC, 1), np.float32), np.ones((1, 1), np.float32)], 0
    )
    wa_aug = np.concatenate([wa_blk, ones_blk], 1).astype(ml_dtypes.bfloat16)
    wo = np.asarray(wo, np.float32)
    wox_aug = np.concatenate(
        [wo[:, :C].T, np.asarray(bo)[None, :]], 0
    ).astype(np.float32)
    woa_t = np.ascontiguousarray(wo[:, C:].T).astype(np.float32)

    in_maps = []
    for core in range(NCORES):
        n, c = divmod(core, 4)
        x_chunk = xf[n][:, c * QPC : (c + 1) * QPC]
        x_aug = np.concatenate([x_chunk, ones_hw[:, :QPC]], 0)
        fm_aug = np.concatenate([fmf[n], ones_hw], 0).astype(ml_dtypes.bfloat16)
        in_maps.append(
            {
                "x_aug": np.ascontiguousarray(x_aug),
                "x_bf": np.ascontiguousarray(x_aug.astype(ml_dtypes.bfloat16)),
                "fm_aug": np.ascontiguousarray(fm_aug),
                "wq_aug": wq_aug,
                "wk_aug": wk_aug,
                "wa_aug": wa_aug,
                "wox_aug": wox_aug,
                "woa_t": woa_t,
            }
        )
    return in_maps


def run(in_maps, trace=False, tmpdir=None):
    nc = _get_nc()
    return bass_utils.run_bass_kernel_spmd(
        nc,
        in_maps,
        core_ids=list(range(NCORES)),
        trace=trace,
        tmpdir=tmpdir,
    )


def kernel(**inputs):
    in_maps = make_in_maps(**inputs)
    res = run(in_maps)
    out = np.empty((N, C, HW), np.float32)
    for core in range(NCORES):
        n, c = divmod(core, 4)
        out[n][:, c * QPC : (c + 1) * QPC] = res.results[core]["out_c"]
    return out.reshape(N, C, H, W)


if __name__ == "__main__":
    import reference

    inputs = {k: np.asarray(v) for k, v in reference.setup_inputs().items()}
    got = kernel(**inputs)
    print("kernel output", got.shape, got.dtype)
